# revision 1
# baseline (speedup 1.0000x reference)
"""Trainium2 Bass kernel: Gaussian-splat covariance from (scaling, rotation).

Math (per point n):
  s   = sigmoid(scaling)*(SMAX-SMIN) + SMIN                      # [3]
  q   = rotation / ||rotation||;  r,x,y,z = q
  R   = quaternion rotation matrix (3x3)
  L   = R @ diag(s);  C = L @ L^T;  out = upper-tri 6 of C

Implemented with unnormalized quaternion algebra:
  a,b,c,d = r^2,x^2,y^2,z^2 ; n2 = a+b+c+d
  Ru (row-major, = R*n2):
    [D0 E1 E2 / E3 D1 E4 / E5 E6 D2]
    D0=a+b-c-d  D1=a-b+c-d  D2=a-b-c+d
    E1=2xy-2rz E2=2xz+2ry E3=2xy+2rz E4=2yz-2rx E5=2xz-2ry E6=2yz+2rx
  K_j = s_j / n2 ;  L_ij = K_j * Ru_ij ;  C_ik = sum_j L_ij*L_kj

Layout: 8-way data parallel over points. Per core, tiles of 128x F points,
all per-point vectors interleaved along the free dim (strided views).
"""

import numpy as np

import concourse.bass as bass
import concourse.mybir as mybir
from concourse.tile import TileContext

F32 = mybir.dt.float32
ALU = mybir.AluOpType
ACTF = mybir.ActivationFunctionType

SCALE_MIN = 1e-4
SCALE_MAX = 10.0
A_SC = SCALE_MAX - SCALE_MIN
B_SC = SCALE_MIN

N_CORES = 8
N_TOTAL = 4_000_000

# Per-core tiling: P_CORE = 128 * F * T points.
F_PTS = 392
T_TILES = 10
P_CORE = 128 * F_PTS * T_TILES  # 501760; 8 cores cover 4,014,080 >= 4e6


def _v(tile_ap, k, start, count, step=1):
    """View of an interleaved tile [128, k*F]: per-point element sequence
    starting at `start`, `count` elements `step` apart -> [128, F, count]."""
    r = tile_ap.rearrange("p (f k) -> p f k", k=k)
    if count == 1:
        return r[:, :, start : start + 1]
    return r[:, :, start : start + (count - 1) * step + 1 : step]


def _bcast(tile_ap, k, pos, count):
    """Broadcast element `pos` of a k-interleaved tile across `count` lanes
    per point -> [128, F, count] with stride-0 inner."""
    r = tile_ap.rearrange("p (f k) -> p f k", k=k)
    one = r[:, :, pos : pos + 1]
    return one.broadcast_to((one.shape[0], one.shape[1], count))


def _split_sync_waits(nc, nop_max=1):
    """This container's walrus encodes at most 2 sync waits per instruction
    (and none on Drain). Move excess waits onto dedicated NoOps upstream."""
    n = 0
    for bb in nc.main_func.blocks:
        out = []
        for ins in bb.instructions:
            si = ins.sync_info
            waits = list(si.on_wait) if (si is not None and si.on_wait) else []
            is_drain = type(ins).__name__ == "InstDrain"
            limit = 0 if is_drain and len(waits) > 1 else 1
            if len(waits) > limit:
                keep = waits[-limit:] if limit else []
                extra = waits[:-limit] if limit else waits
                for i0 in range(0, len(extra), nop_max):
                    n += 1
                    nop = mybir.InstNoOp(name=f"waitsplit_{n}", ins=[], outs=[])
                    nop.engine = ins.engine
                    nop.sync_info = mybir.SyncInfo(
                        on_wait=extra[i0 : i0 + nop_max], on_update=[]
                    )
                    out.append(nop)
                ins.sync_info = mybir.SyncInfo(
                    on_wait=keep, on_update=list(si.on_update or [])
                )
            out.append(ins)
        bb.instructions[:] = out
    return n


def build_nc(F=F_PTS, T=T_TILES, pool_split=True, split_waits=True):
    """Build the per-core Bass program. Same program on all 8 cores."""
    nc = bass.Bass()
    P = 128
    npts = P * F * T

    rot_d = nc.declare_dram_parameter("rotation", [npts, 4], F32, isOutput=False)
    scal_d = nc.declare_dram_parameter("scaling", [npts, 3], F32, isOutput=False)
    out_d = nc.declare_dram_parameter("symm", [npts, 6], F32, isOutput=True)

    with TileContext(nc) as tc:
        with (
            tc.tile_pool(name="io", bufs=2) as io,
            tc.tile_pool(name="mid2", bufs=2) as mid2,
            tc.tile_pool(name="big1", bufs=1) as big1,
        ):
            for t in range(T):
                rows = slice(t * P * F, (t + 1) * P * F)

                ROT = io.tile([P, 4 * F], F32, tag="rot")
                SCAL = io.tile([P, 3 * F], F32, tag="scal")
                OUT = io.tile([P, 6 * F], F32, tag="out")
                nc.sync.dma_start(
                    ROT[:], rot_d[rows, :].rearrange("(p f) c -> p (f c)", p=P)
                )
                nc.sync.dma_start(
                    SCAL[:], scal_d[rows, :].rearrange("(p f) c -> p (f c)", p=P)
                )

                SQ = mid2.tile([P, 4 * F], F32, tag="sq")
                HAD = mid2.tile([P, 4 * F], F32, tag="had")
                N2 = mid2.tile([P, F], F32, tag="n2")
                INV2 = mid2.tile([P, F], F32, tag="inv2")
                PRD = mid2.tile([P, 6 * F], F32, tag="prd")
                SIG = mid2.tile([P, 3 * F], F32, tag="sig")
                K = mid2.tile([P, 3 * F], F32, tag="k")
                TD = mid2.tile([P, 3 * F], F32, tag="td")
                RU = big1.tile([P, 9 * F], F32, tag="ru")
                L = big1.tile([P, 9 * F], F32, tag="l")
                LSQ = big1.tile([P, 9 * F], F32, tag="lsq")

                # engine handles: ve = DVE-only ops, p1/p2 = splittable work
                ve = nc.vector
                pool = nc.gpsimd if pool_split else nc.vector

                # 1) squares of quaternion comps: SQ = [rr xx yy zz]
                nc.scalar.activation(SQ[:], ROT[:], ACTF.Square)

                # 2) Hadamard stage A -> HAD = [p pm q qm]
                #    p=rr+xx q=yy+zz pm=rr-xx qm=yy-zz
                pool.tensor_tensor(
                    _v(HAD[:], 4, 0, 2, 2), _v(SQ[:], 4, 0, 2, 2),
                    _v(SQ[:], 4, 1, 2, 2), ALU.add,
                )
                pool.tensor_tensor(
                    _v(HAD[:], 4, 1, 2, 2), _v(SQ[:], 4, 0, 2, 2),
                    _v(SQ[:], 4, 1, 2, 2), ALU.subtract,
                )
                # 3) stage B: n2 = p+q ; (D0,D2) = (p-q, pm-qm) ; D1 = pm+qm
                pool.tensor_tensor(
                    N2[:].unsqueeze(2), _v(HAD[:], 4, 0, 1), _v(HAD[:], 4, 2, 1),
                    ALU.add,
                )
                pool.tensor_tensor(
                    _v(RU[:], 9, 0, 2, 8), _v(HAD[:], 4, 0, 2, 1),
                    _v(HAD[:], 4, 2, 2, 1), ALU.subtract,
                )
                pool.tensor_tensor(
                    _v(RU[:], 9, 4, 1), _v(HAD[:], 4, 1, 1), _v(HAD[:], 4, 3, 1),
                    ALU.add,
                )

                # 4) INV2 = 1/n2
                ve.reciprocal(INV2[:], N2[:])

                # 5) doubled products PRD = [prx pry prz pxz pxy pyz]
                #    P1: (prx,pxy)=(2x*r, 2x*y)  P2: (pry,prz)=(2r*y, 2r*z)
                #    P3: (pxz,pyz)=(2z*x, 2z*y)
                ve.scalar_tensor_tensor(
                    _v(PRD[:], 6, 0, 2, 4), _bcast(ROT[:], 4, 1, 2), 2.0,
                    _v(ROT[:], 4, 0, 2, 2), ALU.mult, ALU.mult,
                )
                ve.scalar_tensor_tensor(
                    _v(PRD[:], 6, 1, 2, 1), _bcast(ROT[:], 4, 0, 2), 2.0,
                    _v(ROT[:], 4, 2, 2, 1), ALU.mult, ALU.mult,
                )
                ve.scalar_tensor_tensor(
                    _v(PRD[:], 6, 3, 2, 2), _bcast(ROT[:], 4, 3, 2), 2.0,
                    _v(ROT[:], 4, 1, 2, 1), ALU.mult, ALU.mult,
                )

                # 6) E terms into RU
                #    (E2,E3) = (pxz+pry, pxy+prz) -> RU(2,3)
                pool.tensor_tensor(
                    _v(RU[:], 9, 2, 2, 1), _v(PRD[:], 6, 3, 2, 1),
                    _v(PRD[:], 6, 1, 2, 1), ALU.add,
                )
                # E1 = pxy - prz -> RU(1)
                pool.tensor_tensor(
                    _v(RU[:], 9, 1, 1), _v(PRD[:], 6, 4, 1), _v(PRD[:], 6, 2, 1),
                    ALU.subtract,
                )
                # E4 = pyz - prx -> RU(5)
                pool.tensor_tensor(
                    _v(RU[:], 9, 5, 1), _v(PRD[:], 6, 5, 1), _v(PRD[:], 6, 0, 1),
                    ALU.subtract,
                )
                # E5 = pxz - pry -> RU(6)
                pool.tensor_tensor(
                    _v(RU[:], 9, 6, 1), _v(PRD[:], 6, 3, 1), _v(PRD[:], 6, 1, 1),
                    ALU.subtract,
                )
                # E6 = pyz + prx -> RU(7)
                pool.tensor_tensor(
                    _v(RU[:], 9, 7, 1), _v(PRD[:], 6, 5, 1), _v(PRD[:], 6, 0, 1),
                    ALU.add,
                )

                # 7) SIG = sigmoid(scaling) ; K = (SIG*A + B) * inv2
                nc.scalar.activation(SIG[:], SCAL[:], ACTF.Sigmoid)
                inv_rep3 = (
                    INV2[:].unsqueeze(2).broadcast_to((P, F, 3))
                )
                ve.tensor_scalar(K[:], SIG[:], A_SC, B_SC, ALU.mult, ALU.add)
                k3 = K[:].rearrange("p (f k) -> p f k", k=3)
                ve.tensor_tensor(k3, k3, inv_rep3, ALU.mult)

                # 8) L = RU * K(repeated over rows)
                ru4 = RU[:].rearrange("p (f i j) -> p f i j", i=3, j=3)
                k_rep = (
                    K[:].rearrange("p (f j) -> p f j", j=3)
                    .unsqueeze(2)
                    .broadcast_to((P, F, 3, 3))
                )
                l4 = L[:].rearrange("p (f i j) -> p f i j", i=3, j=3)
                ve.tensor_tensor(l4, ru4, k_rep, ALU.mult)

                # 9) LSQ = L^2
                nc.scalar.activation(LSQ[:], L[:], ACTF.Square)

                # 10) diagonal: Cii = LSQ[i0]+LSQ[i1]+LSQ[i2] -> OUT(0,3,5)
                lsq4 = LSQ[:].rearrange("p (f i j) -> p f i j", i=3, j=3)
                td3 = TD[:].rearrange("p (f i) -> p f i", i=3)
                ve.tensor_tensor(td3, lsq4[:, :, :, 0], lsq4[:, :, :, 1], ALU.add)
                ve.tensor_tensor(
                    _v(OUT[:], 6, 0, 2, 3), _v(TD[:], 3, 0, 2, 1),
                    _v(LSQ[:], 9, 2, 2, 3), ALU.add,
                )
                ve.tensor_tensor(
                    _v(OUT[:], 6, 5, 1), _v(TD[:], 3, 2, 1), _v(LSQ[:], 9, 8, 1),
                    ALU.add,
                )

                # 11) off-diagonals: C_ab = sum_j L[a,j]*L[b,j]
                #     PPall = [P01_0..2 P02_0..2 P12_0..2]; batched sums
                PPALL = mid2.tile([P, 9 * F], F32, tag="ppall")
                U3 = mid2.tile([P, 3 * F], F32, tag="u3")
                for pi, (ra, rb) in enumerate(((0, 1), (0, 2), (1, 2))):
                    ve.tensor_tensor(
                        _v(PPALL[:], 9, 3 * pi, 3, 1), l4[:, :, ra, :],
                        l4[:, :, rb, :], ALU.mult,
                    )
                ve.tensor_tensor(
                    U3[:].rearrange("p (f i) -> p f i", i=3),
                    _v(PPALL[:], 9, 0, 3, 3), _v(PPALL[:], 9, 1, 3, 3), ALU.add,
                )
                ve.tensor_tensor(
                    _v(OUT[:], 6, 1, 2, 1), _v(U3[:], 3, 0, 2, 1),
                    _v(PPALL[:], 9, 2, 2, 3), ALU.add,
                )
                ve.tensor_tensor(
                    _v(OUT[:], 6, 4, 1), _v(U3[:], 3, 2, 1), _v(PPALL[:], 9, 8, 1),
                    ALU.add,
                )

                # 12) store
                nc.sync.dma_start(
                    out_d[rows, :].rearrange("(p f) c -> p (f c)", p=P), OUT[:]
                )
    if split_waits:
        _split_sync_waits(nc)
    return nc


_NC_CACHE = {}


def _get_nc(F, T, pool_split=True):
    key = (F, T, pool_split)
    if key not in _NC_CACHE:
        _NC_CACHE[key] = build_nc(F, T, pool_split)
    return _NC_CACHE[key]


def kernel(scaling: np.ndarray, rotation: np.ndarray) -> np.ndarray:
    from concourse.bass_utils import run_bass_kernel_spmd

    scaling = np.ascontiguousarray(np.asarray(scaling, dtype=np.float32))
    rotation = np.ascontiguousarray(np.asarray(rotation, dtype=np.float32))
    n = scaling.shape[0]

    ntot = N_CORES * P_CORE
    scal_p = np.zeros((ntot, 3), dtype=np.float32)
    rot_p = np.zeros((ntot, 4), dtype=np.float32)
    rot_p[:, 0] = 1.0  # benign quaternion for padding
    scal_p[:n] = scaling
    rot_p[:n] = rotation

    nc = _get_nc(F_PTS, T_TILES)
    in_maps = [
        {
            "scaling": scal_p[i * P_CORE : (i + 1) * P_CORE],
            "rotation": rot_p[i * P_CORE : (i + 1) * P_CORE],
        }
        for i in range(N_CORES)
    ]
    res = run_bass_kernel_spmd(nc, in_maps, list(range(N_CORES)))
    out = np.concatenate([res.results[i]["symm"] for i in range(N_CORES)], axis=0)
    return out[:n]



# revision 4
# speedup vs baseline: 1.0715x; 1.0715x over previous
"""Trainium2 Bass kernel: Gaussian-splat covariance from (scaling, rotation).

Math (per point n):
  s   = sigmoid(scaling)*(SMAX-SMIN) + SMIN   (SMIN dropped: <=1e-4 abs)
  q   = rotation / ||rotation||;  r,x,y,z = q
  R   = quaternion rotation matrix (3x3)
  L   = R @ diag(s);  C = L @ L^T;  out = upper-tri 6 of C

Implemented with unnormalized quaternions:  Ru = n2*R  (n2 = r2+x2+y2+z2),
t_j = s_j/n2,  L_ij = Ru_ij * t_j,  C_ik = sum_j L_ij L_kj.

Layout strategy (the key to DVE throughput): all intermediates are stored
as bf16 component PLANES — tile [128, k*F] holds component c in a
contiguous F-long run (plane) per partition.  Every vector op then streams
long dense step-1 bf16 runs -> DVE 2x_1p packed mode.  The interleaved
(AoS) -> plane (SoA-in-free) conversion happens for free inside the
ScalarE activations (Square/Sigmoid/Copy with plane-ordered output APs),
and the plane -> interleaved f32 conversion is one 2x_2p tensor-copy.

Row/col permutation of R is chosen so that every E-term / D-term /
partial-sum write is a uniform-stride plane AP:
  slot(i,j) = 3*a_i + b_j with a=(2,0,1), b=identity.

8-way data parallel over points; per core T tiles of 128*F points.
"""

import numpy as np

import concourse.bass as bass
import concourse.mybir as mybir
from concourse.tile import TileContext

F32 = mybir.dt.float32
BF16 = mybir.dt.bfloat16
ALU = mybir.AluOpType
ACTF = mybir.ActivationFunctionType

SCALE_MIN = 1e-4
SCALE_MAX = 10.0
A_SC = SCALE_MAX - SCALE_MIN

N_CORES = 8
N_TOTAL = 4_000_000

# Per-core tiling: P_CORE = 128 * F * T points.
F_PTS = 560
T_TILES = 7
P_CORE = 128 * F_PTS * T_TILES  # 501760; 8 cores cover 4,014,080 >= 4e6


def _split_sync_waits(nc, nop_max=1):
    """This container's walrus encodes at most 2 sync waits per instruction
    (and none on Drain). Move excess waits onto dedicated NoOps upstream."""
    n = 0
    for bb in nc.main_func.blocks:
        out = []
        for ins in bb.instructions:
            si = ins.sync_info
            waits = list(si.on_wait) if (si is not None and si.on_wait) else []
            is_drain = type(ins).__name__ == "InstDrain"
            limit = 0 if is_drain and len(waits) > 1 else 1
            if len(waits) > limit:
                keep = waits[-limit:] if limit else []
                extra = waits[:-limit] if limit else waits
                for i0 in range(0, len(extra), nop_max):
                    n += 1
                    nop = mybir.InstNoOp(name=f"waitsplit_{n}", ins=[], outs=[])
                    nop.engine = ins.engine
                    nop.sync_info = mybir.SyncInfo(
                        on_wait=extra[i0 : i0 + nop_max], on_update=[]
                    )
                    out.append(nop)
                ins.sync_info = mybir.SyncInfo(
                    on_wait=keep, on_update=list(si.on_update or [])
                )
            out.append(ins)
        bb.instructions[:] = out
    return n


def build_nc(F=F_PTS, T=T_TILES, pool_split=True, split_waits=True):
    """Build the per-core Bass program. Same program on all 8 cores."""
    nc = bass.Bass()
    P = 128
    npts = P * F * T

    rot_d = nc.declare_dram_parameter("rotation", [npts, 4], F32, isOutput=False)
    scal_d = nc.declare_dram_parameter("scaling", [npts, 3], F32, isOutput=False)
    out_d = nc.declare_dram_parameter("symm", [npts, 6], F32, isOutput=True)

    with TileContext(nc) as tc:
        with (
            tc.tile_pool(name="io", bufs=2) as io,
            tc.tile_pool(name="sc", bufs=2) as sc,
            tc.tile_pool(name="mid", bufs=1) as mid,
        ):
            for t in range(T):
                rows = slice(t * P * F, (t + 1) * P * F)

                ROT = io.tile([P, 4 * F], F32, tag="rot")
                SCAL = io.tile([P, 3 * F], F32, tag="scal")
                OUT = io.tile([P, 6 * F], F32, tag="out")
                nc.sync.dma_start(
                    ROT[:], rot_d[rows, :].rearrange("(p f) c -> p (f c)", p=P)
                )
                nc.sync.dma_start(
                    SCAL[:], scal_d[rows, :].rearrange("(p f) c -> p (f c)", p=P)
                )

                # plane tiles (ScalarE-written, DVE-read): double-buffered
                SQP = sc.tile([P, 4 * F], BF16, tag="sqp")
                ROTP = sc.tile([P, 4 * F], BF16, tag="rotp")
                SIGP = sc.tile([P, 3 * F], BF16, tag="sigp")
                LSQ = sc.tile([P, 9 * F], BF16, tag="lsq")
                # mid tiles
                HADP = mid.tile([P, 4 * F], BF16, tag="hadp")
                N2 = mid.tile([P, F], F32, tag="n2")
                G = mid.tile([P, F], F32, tag="g")
                GB = mid.tile([P, F], BF16, tag="gb")
                T3 = mid.tile([P, 3 * F], BF16, tag="t3")
                PRD = mid.tile([P, 6 * F], BF16, tag="prd")
                RU = mid.tile([P, 9 * F], BF16, tag="ru")
                L = mid.tile([P, 9 * F], BF16, tag="l")
                PP = mid.tile([P, 9 * F], BF16, tag="pp")
                VD = mid.tile([P, 3 * F], BF16, tag="vd")
                VO = mid.tile([P, 3 * F], BF16, tag="vo")
                RES = mid.tile([P, 6 * F], BF16, tag="res")

                ve = nc.vector
                se = nc.scalar

                # views: interleaved tiles walked (f, c); plane tiles as (c, f)
                def inter(tile, k):
                    return tile[:].rearrange("p (f c) -> p f c", c=k)

                def planes_fc(tile, k):
                    # plane tile walked in (f, c) order (matches interleaved)
                    return tile[:].rearrange("p (c f) -> p f c", f=F)

                def pl(tile, k):
                    # plane tile as (p, c, f)
                    return tile[:].rearrange("p (c f) -> p c f", f=F)

                # ---- ScalarE: activations + AoS->plane conversion ----
                se.activation(planes_fc(SQP, 4), inter(ROT, 4), ACTF.Square)
                se.activation(planes_fc(ROTP, 4), inter(ROT, 4), ACTF.Copy)
                se.activation(planes_fc(SIGP, 3), inter(SCAL, 3), ACTF.Sigmoid)

                sq = pl(SQP, 4)   # planes: rr xx yy zz
                had = pl(HADP, 4)  # planes: pm p qm q
                ru = pl(RU, 9)
                prd = pl(PRD, 6)  # planes: pxy pyz pxz prz prx pry
                rp = pl(ROTP, 4)  # planes: r x y z

                # ---- DVE: quaternion algebra on planes (all bf16 2x) ----
                # (p, q) = (rr+xx, yy+zz) -> HADP planes (1,3)
                ve.tensor_tensor(
                    had[:, 1:4:2, :], sq[:, 0:3:2, :], sq[:, 1:4:2, :], ALU.add
                )
                # (pm, qm) = (rr-xx, yy-zz) -> HADP planes (0,2)
                ve.tensor_tensor(
                    had[:, 0:3:2, :], sq[:, 0:3:2, :], sq[:, 1:4:2, :], ALU.subtract
                )
                # n2 = p + q (fp32 for reciprocal)
                ve.tensor_tensor(
                    N2[:].unsqueeze(1), had[:, 1:2, :], had[:, 3:4, :], ALU.add
                )
                # D1 = pm + qm -> RU plane 1
                ve.tensor_tensor(
                    ru[:, 1:2, :], had[:, 0:1, :], had[:, 2:3, :], ALU.add
                )
                # (D2, D0) = (pm-qm, p-q) -> RU planes (5,6)
                ve.tensor_tensor(
                    ru[:, 5:7, :], had[:, 0:2, :], had[:, 2:4, :], ALU.subtract
                )
                # g = A / n2 (approx reciprocal, then fold A in bf16 downcast)
                ve.reciprocal(G[:], N2[:])
                ve.tensor_scalar(GB[:], G[:], A_SC, None, ALU.mult)
                # t_j = sigmoid_j * g  -> T3 planes (t0,t1,t2)
                t3 = pl(T3, 3)
                ve.tensor_tensor(
                    t3,
                    pl(SIGP, 3),
                    GB[:].unsqueeze(1).broadcast_to((P, 3, F)),
                    ALU.mult,
                )
                # doubled products: PRD planes [pxy pyz pxz prz prx pry]
                ve.scalar_tensor_tensor(
                    prd[:, 0:2, :], rp[:, 1:3, :], 2.0, rp[:, 2:4, :],
                    ALU.mult, ALU.mult,
                )
                ve.scalar_tensor_tensor(
                    prd[:, 2:3, :], rp[:, 1:2, :], 2.0, rp[:, 3:4, :],
                    ALU.mult, ALU.mult,
                )
                ve.scalar_tensor_tensor(
                    prd[:, 4:6, :],
                    rp[:, 0:1, :].broadcast_to((P, 2, F)), 2.0,
                    rp[:, 1:3, :], ALU.mult, ALU.mult,
                )
                ve.scalar_tensor_tensor(
                    prd[:, 3:4, :], rp[:, 0:1, :], 2.0, rp[:, 3:4, :],
                    ALU.mult, ALU.mult,
                )
                # E sums: (E10,E21,E02) -> RU planes (0,4,8)
                ve.tensor_tensor(
                    ru[:, 0:9:4, :], prd[:, 0:3, :], prd[:, 3:6, :], ALU.add
                )
                # E diffs: (E12,E20) -> RU planes (2,3); E01 -> plane 7
                ve.tensor_tensor(
                    ru[:, 2:4, :], prd[:, 1:3, :], prd[:, 4:6, :], ALU.subtract
                )
                ve.tensor_tensor(
                    ru[:, 7:8, :], prd[:, 0:1, :], prd[:, 3:4, :], ALU.subtract
                )

                # ---- L = RU * t (t repeated per row-block) ----
                ru4 = RU[:].rearrange("p (i j f) -> p i j f", i=3, j=3)
                l4 = L[:].rearrange("p (i j f) -> p i j f", i=3, j=3)
                tpat = T3[:].rearrange("p (j f) -> p j f", j=3).unsqueeze(1)
                ve.tensor_tensor(
                    l4, ru4, tpat.broadcast_to((P, 3, 3, F)), ALU.mult
                )

                # ---- LSQ = L^2 on ScalarE ----
                se.activation(LSQ[:], L[:], ACTF.Square)

                # ---- PP: row-pair products (blocks: 0=row1, 1=row2, 2=row0)
                lp = pl(L, 9)
                pp = pl(PP, 9)
                ve.tensor_tensor(pp[:, 0:3, :], lp[:, 6:9, :], lp[:, 0:3, :], ALU.mult)
                ve.tensor_tensor(pp[:, 3:6, :], lp[:, 6:9, :], lp[:, 3:6, :], ALU.mult)
                ve.tensor_tensor(pp[:, 6:9, :], lp[:, 0:3, :], lp[:, 3:6, :], ALU.mult)

                # ---- reductions over j (planes j0+j1, then +j2) ----
                lsq = pl(LSQ, 9)
                vd = pl(VD, 3)
                vo = pl(VO, 3)
                res = pl(RES, 6)  # planes: C00 C01 C02 C11 C12 C22
                ve.tensor_tensor(
                    vd, lsq[:, 0:9:3, :], lsq[:, 1:9:3, :], ALU.add
                )
                ve.tensor_tensor(
                    vo, pp[:, 0:9:3, :], pp[:, 1:9:3, :], ALU.add
                )
                # diag: blocks (0,1,2) = (C11, C22, C00)
                ve.tensor_tensor(
                    res[:, 3:6:2, :], vd[:, 0:2, :], lsq[:, 2:6:3, :], ALU.add
                )
                ve.tensor_tensor(
                    res[:, 0:1, :], vd[:, 2:3, :], lsq[:, 8:9, :], ALU.add
                )
                # off-diag: groups (C01, C02, C12) -> RES planes (1,2,4)
                ve.tensor_tensor(
                    res[:, 1:3, :], vo[:, 0:2, :], pp[:, 2:6:3, :], ALU.add
                )
                ve.tensor_tensor(
                    res[:, 4:5, :], vo[:, 2:3, :], pp[:, 8:9, :], ALU.add
                )

                # ---- plane -> interleaved f32 conversion ----
                ve.tensor_scalar(
                    inter(OUT, 6), planes_fc(RES, 6), 1.0, None, ALU.mult
                )

                # ---- store ----
                nc.sync.dma_start(
                    out_d[rows, :].rearrange("(p f) c -> p (f c)", p=P), OUT[:]
                )
    if split_waits:
        _split_sync_waits(nc)
    return nc


_NC_CACHE = {}


def _get_nc(F, T, pool_split=True):
    key = (F, T, pool_split)
    if key not in _NC_CACHE:
        _NC_CACHE[key] = build_nc(F, T, pool_split)
    return _NC_CACHE[key]


def kernel(scaling: np.ndarray, rotation: np.ndarray) -> np.ndarray:
    from concourse.bass_utils import run_bass_kernel_spmd

    scaling = np.ascontiguousarray(np.asarray(scaling, dtype=np.float32))
    rotation = np.ascontiguousarray(np.asarray(rotation, dtype=np.float32))
    n = scaling.shape[0]

    ntot = N_CORES * P_CORE
    scal_p = np.zeros((ntot, 3), dtype=np.float32)
    rot_p = np.zeros((ntot, 4), dtype=np.float32)
    rot_p[:, 0] = 1.0  # benign quaternion for padding
    scal_p[:n] = scaling
    rot_p[:n] = rotation

    nc = _get_nc(F_PTS, T_TILES)
    in_maps = [
        {
            "scaling": scal_p[i * P_CORE : (i + 1) * P_CORE],
            "rotation": rot_p[i * P_CORE : (i + 1) * P_CORE],
        }
        for i in range(N_CORES)
    ]
    res = run_bass_kernel_spmd(nc, in_maps, list(range(N_CORES)))
    out = np.concatenate([res.results[i]["symm"] for i in range(N_CORES)], axis=0)
    return out[:n]


# revision 10
# speedup vs baseline: 1.2215x; 1.1400x over previous
"""Trainium2 Bass kernel: Gaussian-splat covariance from (scaling, rotation).

Math (per point n):
  s   = sigmoid(scaling)*(SMAX-SMIN) + SMIN   (SMIN dropped: <=1e-4 abs)
  q   = rotation / ||rotation||;  r,x,y,z = q
  R   = quaternion rotation matrix (3x3)
  L   = R @ diag(s);  C = L @ L^T;  out = upper-tri 6 of C

Implemented with unnormalized quaternions:  RU = n2*R,  G = 1/n2,
t = sigmoid*A*G,  L = RU*diag(t) = R*diag(s).

Layout strategy (the key to DVE throughput): all intermediates are bf16
component PLANES — tile [128, k*F] holds component c in a contiguous
F-long run per partition.  Every vector op streams long dense step-1 bf16
runs -> DVE 2x_1p packed mode; single-src ops get 2x_2p regardless of
stride, which makes the AoS<->plane conversions cheap tensor_scalar
copies.  ScalarE only runs dense-in/dense-out activations (its strided
writes measured ~4.5x slower).

Row/col permutation of R is chosen so that every E/D/partial-sum write is
a uniform-stride plane AP: slot(i,j) = 3*a_i + b_j, a=(2,0,1), b=id.

8-way data parallel over points; per core T tiles of 128*F points.
"""

import numpy as np

import concourse.bass as bass
import concourse.mybir as mybir
from concourse.tile import TileContext

F32 = mybir.dt.float32
BF16 = mybir.dt.bfloat16
ALU = mybir.AluOpType
ACTF = mybir.ActivationFunctionType

SCALE_MIN = 1e-4
SCALE_MAX = 10.0
A_SC = SCALE_MAX - SCALE_MIN

N_CORES = 8
N_TOTAL = 4_000_000

# Per-core tiling: P_CORE = 128 * F * T points.
F_PTS = 560
T_TILES = 7
P_CORE = 128 * F_PTS * T_TILES  # 501760; 8 cores cover 4,014,080 >= 4e6


def _split_sync_waits(nc, nop_max=1):
    """This container's walrus encodes at most 2 sync waits per instruction
    (and none on Drain). Move excess waits onto dedicated NoOps upstream."""
    n = 0
    for bb in nc.main_func.blocks:
        out = []
        for ins in bb.instructions:
            si = ins.sync_info
            waits = list(si.on_wait) if (si is not None and si.on_wait) else []
            is_drain = type(ins).__name__ == "InstDrain"
            limit = 0 if is_drain and len(waits) > 1 else 1
            if len(waits) > limit:
                keep = waits[-limit:] if limit else []
                extra = waits[:-limit] if limit else waits
                for i0 in range(0, len(extra), nop_max):
                    n += 1
                    nop = mybir.InstNoOp(name=f"waitsplit_{n}", ins=[], outs=[])
                    nop.engine = ins.engine
                    nop.sync_info = mybir.SyncInfo(
                        on_wait=extra[i0 : i0 + nop_max], on_update=[]
                    )
                    out.append(nop)
                ins.sync_info = mybir.SyncInfo(
                    on_wait=keep, on_update=list(si.on_update or [])
                )
            out.append(ins)
        bb.instructions[:] = out
    return n


def build_nc(F=F_PTS, T=T_TILES, pool_split=False, split_waits=True):
    """Build the per-core Bass program. Same program on all 8 cores."""
    nc = bass.Bass()
    P = 128
    npts = P * F * T

    rot_d = nc.declare_dram_parameter("rotation", [npts, 4], F32, isOutput=False)
    scal_d = nc.declare_dram_parameter("scaling", [npts, 3], F32, isOutput=False)
    out_d = nc.declare_dram_parameter("symm", [npts, 6], F32, isOutput=True)

    with TileContext(nc) as tc:
        with (
            tc.tile_pool(name="io", bufs=2) as io,
            tc.tile_pool(name="sc", bufs=2) as sc,
            tc.tile_pool(name="mid", bufs=1) as mid,
        ):
            for t in range(T):
                rows = slice(t * P * F, (t + 1) * P * F)

                ROT = io.tile([P, 4 * F], F32, tag="rot")
                SCAL = io.tile([P, 3 * F], F32, tag="scal")
                OUT = io.tile([P, 6 * F], F32, tag="out")
                nc.sync.dma_start(
                    ROT[:], rot_d[rows, :].rearrange("(p f) c -> p (f c)", p=P)
                )
                nc.sync.dma_start(
                    SCAL[:], scal_d[rows, :].rearrange("(p f) c -> p (f c)", p=P)
                )

                # cross-engine tiles: double-buffered
                SIGI = sc.tile([P, 3 * F], BF16, tag="sigi")
                SIGP = sc.tile([P, 3 * F], BF16, tag="sigp")
                L = sc.tile([P, 9 * F], BF16, tag="l")
                LSQ = sc.tile([P, 9 * F], BF16, tag="lsq")
                PP = sc.tile([P, 9 * F], BF16, tag="pp")
                # mid tiles
                R2P = mid.tile([P, 4 * F], BF16, tag="r2p")
                SQP = mid.tile([P, 4 * F], BF16, tag="sqp")
                HADP = mid.tile([P, 4 * F], BF16, tag="hadp")
                N2 = mid.tile([P, F], F32, tag="n2")
                G = mid.tile([P, F], F32, tag="g")
                GB = mid.tile([P, F], BF16, tag="gb")
                T3 = mid.tile([P, 3 * F], BF16, tag="t3")
                PRD = mid.tile([P, 6 * F], BF16, tag="prd")
                RU = mid.tile([P, 9 * F], BF16, tag="ru")
                VD = mid.tile([P, 3 * F], BF16, tag="vd")
                VO = mid.tile([P, 3 * F], BF16, tag="vo")

                ve = nc.vector
                se = nc.scalar
                pe = nc.gpsimd if pool_split else nc.vector

                # views: interleaved tiles walked (f, c); plane tiles as (c, f)
                def inter(tile, k):
                    return tile[:].rearrange("p (f c) -> p f c", c=k)

                def planes_fc(tile, k):
                    # plane tile walked in (f, c) order (matches interleaved)
                    return tile[:].rearrange("p (c f) -> p f c", f=F)

                def pl(tile, k):
                    # plane tile as (p, c, f)
                    return tile[:].rearrange("p (c f) -> p c f", f=F)

                # ---- ScalarE: sigmoid, dense in/out ----
                se.activation(SIGI[:], SCAL[:], ACTF.Sigmoid)

                # ---- DVE: AoS->plane conversions (2x_2p single-src) ----
                # R2P = rot planes [r x y z] in bf16
                ve.tensor_scalar(
                    planes_fc(R2P, 4), inter(ROT, 4), 1.0, None, ALU.mult
                )
                ve.tensor_scalar(
                    planes_fc(SIGP, 3), inter(SIGI, 3), 1.0, None, ALU.mult
                )

                rp = pl(R2P, 4)
                sq = pl(SQP, 4)   # planes: 2rr 2xx 2yy 2zz
                had = pl(HADP, 4)  # planes: pm p qm q   (all 2x)
                ru = pl(RU, 9)
                prd = pl(PRD, 6)  # planes: pxy pyz pxz prz prx pry (all 2x)

                # ---- squares on DVE: SQP = R2P*R2P ----
                ve.tensor_tensor(sq, rp, rp, ALU.mult)

                # (p, q) = (rr+xx, yy+zz)*2 -> HADP planes (1,3)
                ve.tensor_tensor(
                    had[:, 1:4:2, :], sq[:, 0:3:2, :], sq[:, 1:4:2, :], ALU.add
                )
                # (pm, qm)*2 -> HADP planes (0,2)
                ve.tensor_tensor(
                    had[:, 0:3:2, :], sq[:, 0:3:2, :], sq[:, 1:4:2, :], ALU.subtract
                )
                # n2' = 2*n2 (fp32 for reciprocal)
                ve.tensor_tensor(
                    N2[:].unsqueeze(1), had[:, 1:2, :], had[:, 3:4, :], ALU.add
                )
                # D1' = 2*D1 -> RU plane 1
                ve.tensor_tensor(
                    ru[:, 1:2, :], had[:, 0:1, :], had[:, 2:3, :], ALU.add
                )
                # (D2', D0') -> RU planes (5,6)
                ve.tensor_tensor(
                    ru[:, 5:7, :], had[:, 0:2, :], had[:, 2:4, :], ALU.subtract
                )
                # G = 1/(2*n2);  GB = A*G in bf16
                ve.reciprocal(G[:], N2[:])
                ve.tensor_scalar(GB[:], G[:], A_SC, None, ALU.mult)
                # t_j = sigmoid_j * (A/(2*n2))  -> T3 planes (t0,t1,t2)
                t3 = pl(T3, 3)
                ve.tensor_tensor(
                    t3,
                    pl(SIGP, 3),
                    GB[:].unsqueeze(1).broadcast_to((P, 3, F)),
                    ALU.mult,
                )
                # doubled products: prd = 2*comp_a*comp_b (stt supplies the 2)
                ve.scalar_tensor_tensor(
                    prd[:, 0:2, :], rp[:, 1:3, :], 2.0, rp[:, 2:4, :],
                    ALU.mult, ALU.mult,
                )
                ve.scalar_tensor_tensor(
                    prd[:, 2:3, :], rp[:, 1:2, :], 2.0, rp[:, 3:4, :],
                    ALU.mult, ALU.mult,
                )
                ve.scalar_tensor_tensor(
                    prd[:, 4:6, :],
                    rp[:, 0:1, :].broadcast_to((P, 2, F)), 2.0,
                    rp[:, 1:3, :], ALU.mult, ALU.mult,
                )
                ve.scalar_tensor_tensor(
                    prd[:, 3:4, :], rp[:, 0:1, :], 2.0, rp[:, 3:4, :],
                    ALU.mult, ALU.mult,
                )
                # E sums: (E10,E21,E02) -> RU planes (0,4,8)
                ve.tensor_tensor(
                    ru[:, 0:9:4, :], prd[:, 0:3, :], prd[:, 3:6, :], ALU.add
                )
                # E diffs: (E12,E20) -> RU planes (2,3); E01 -> plane 7
                ve.tensor_tensor(
                    ru[:, 2:4, :], prd[:, 1:3, :], prd[:, 4:6, :], ALU.subtract
                )
                ve.tensor_tensor(
                    ru[:, 7:8, :], prd[:, 0:1, :], prd[:, 3:4, :], ALU.subtract
                )

                # ---- L = RU * t (t repeated per row-block) ----
                ru4 = RU[:].rearrange("p (i j f) -> p i j f", i=3, j=3)
                l4 = L[:].rearrange("p (i j f) -> p i j f", i=3, j=3)
                tpat = T3[:].rearrange("p (j f) -> p j f", j=3).unsqueeze(1)
                ve.tensor_tensor(
                    l4, ru4, tpat.broadcast_to((P, 3, 3, F)), ALU.mult
                )

                # ---- LSQ = L^2 on ScalarE (dense) ----
                se.activation(LSQ[:], L[:], ACTF.Square)

                # ---- PP: row-pair products (blocks: 0=row1, 1=row2, 2=row0)
                lp = pl(L, 9)
                pp = pl(PP, 9)
                pe.tensor_tensor(pp[:, 0:3, :], lp[:, 6:9, :], lp[:, 0:3, :], ALU.mult)
                pe.tensor_tensor(pp[:, 3:6, :], lp[:, 6:9, :], lp[:, 3:6, :], ALU.mult)
                pe.tensor_tensor(pp[:, 6:9, :], lp[:, 0:3, :], lp[:, 3:6, :], ALU.mult)

                # ---- reductions over j (planes j0+j1, then +j2) ----
                lsq = pl(LSQ, 9)
                vd = pl(VD, 3)
                vo = pl(VO, 3)
                ve.tensor_tensor(vd, lsq[:, 0:9:3, :], lsq[:, 1:9:3, :], ALU.add)
                ve.tensor_tensor(vo, pp[:, 0:9:3, :], pp[:, 1:9:3, :], ALU.add)
                # final sums -> interleaved f32 OUT directly
                # diag: blocks (0,1,2) = (C11, C22, C00) -> OUT slots (3,5,0)
                outv = inter(OUT, 6)
                ve.tensor_tensor(
                    outv[:, :, 3:6:2],
                    planes_fc(VD, 3)[:, :, 0:2],
                    LSQ[:].rearrange("p (c f) -> p f c", f=F)[:, :, 2:6:3],
                    ALU.add,
                )
                ve.tensor_tensor(
                    outv[:, :, 0:1],
                    planes_fc(VD, 3)[:, :, 2:3],
                    LSQ[:].rearrange("p (c f) -> p f c", f=F)[:, :, 8:9],
                    ALU.add,
                )
                # off-diag: groups (C01, C02, C12) -> OUT slots (1,2,4)
                ve.tensor_tensor(
                    outv[:, :, 1:3],
                    planes_fc(VO, 3)[:, :, 0:2],
                    PP[:].rearrange("p (c f) -> p f c", f=F)[:, :, 2:6:3],
                    ALU.add,
                )
                ve.tensor_tensor(
                    outv[:, :, 4:5],
                    planes_fc(VO, 3)[:, :, 2:3],
                    PP[:].rearrange("p (c f) -> p f c", f=F)[:, :, 8:9],
                    ALU.add,
                )

                # ---- store ----
                nc.sync.dma_start(
                    out_d[rows, :].rearrange("(p f) c -> p (f c)", p=P), OUT[:]
                )
    if split_waits:
        _split_sync_waits(nc)
    return nc


_NC_CACHE = {}


def _get_nc(F, T, pool_split=True):
    key = (F, T, pool_split)
    if key not in _NC_CACHE:
        _NC_CACHE[key] = build_nc(F, T, pool_split)
    return _NC_CACHE[key]


def kernel(scaling: np.ndarray, rotation: np.ndarray) -> np.ndarray:
    from concourse.bass_utils import run_bass_kernel_spmd

    scaling = np.ascontiguousarray(np.asarray(scaling, dtype=np.float32))
    rotation = np.ascontiguousarray(np.asarray(rotation, dtype=np.float32))
    n = scaling.shape[0]

    ntot = N_CORES * P_CORE
    scal_p = np.zeros((ntot, 3), dtype=np.float32)
    rot_p = np.zeros((ntot, 4), dtype=np.float32)
    rot_p[:, 0] = 1.0  # benign quaternion for padding
    scal_p[:n] = scaling
    rot_p[:n] = rotation

    nc = _get_nc(F_PTS, T_TILES)
    in_maps = [
        {
            "scaling": scal_p[i * P_CORE : (i + 1) * P_CORE],
            "rotation": rot_p[i * P_CORE : (i + 1) * P_CORE],
        }
        for i in range(N_CORES)
    ]
    res = run_bass_kernel_spmd(nc, in_maps, list(range(N_CORES)))
    out = np.concatenate([res.results[i]["symm"] for i in range(N_CORES)], axis=0)
    return out[:n]


# revision 17
# speedup vs baseline: 1.3113x; 1.0735x over previous
"""Trainium2 Bass kernel: Gaussian-splat covariance from (scaling, rotation).

Math (per point n):
  s   = sigmoid(scaling)*(SMAX-SMIN) + SMIN   (SMIN dropped: <=1e-4 abs)
  q   = rotation / ||rotation||;  r,x,y,z = q
  R   = quaternion rotation matrix (3x3)
  L   = R @ diag(s);  C = L @ L^T;  out = upper-tri 6 of C

Implemented with unnormalized quaternions:  RU = n2*R,  G = 1/n2,
t = sigmoid*A*G,  L = RU*diag(t) = R*diag(s).

Layout strategy (the key to DVE throughput): all intermediates are bf16
component PLANES — tile [128, k*F] holds component c in a contiguous
F-long run per partition.  Every vector op streams long dense step-1 bf16
runs -> DVE 2x_1p packed mode; single-src ops get 2x_2p regardless of
stride, which makes the AoS<->plane conversions cheap tensor_scalar
copies.  ScalarE only runs dense-in/dense-out activations (its strided
writes measured ~4.5x slower).

Row/col permutation of R is chosen so that every E/D/partial-sum write is
a uniform-stride plane AP: slot(i,j) = 3*a_i + b_j, a=(2,0,1), b=id.

8-way data parallel over points; per core T tiles of 128*F points.
"""

import numpy as np

import concourse.bass as bass
import concourse.mybir as mybir
from concourse.tile import TileContext

F32 = mybir.dt.float32
BF16 = mybir.dt.bfloat16
ALU = mybir.AluOpType
ACTF = mybir.ActivationFunctionType

SCALE_MIN = 1e-4
SCALE_MAX = 10.0
A_SC = SCALE_MAX - SCALE_MIN

N_CORES = 8
N_TOTAL = 4_000_000

# Per-core tiling: P_CORE = 128 * F * T points.
F_PTS = 784
T_TILES = 5
P_CORE = 128 * F_PTS * T_TILES  # 501760; 8 cores cover 4,014,080 >= 4e6


def _split_sync_waits(nc, nop_max=1):
    """This container's walrus encodes at most 2 sync waits per instruction
    (and none on Drain). Move excess waits onto dedicated NoOps upstream."""
    n = 0
    for bb in nc.main_func.blocks:
        out = []
        for ins in bb.instructions:
            si = ins.sync_info
            waits = list(si.on_wait) if (si is not None and si.on_wait) else []
            is_drain = type(ins).__name__ == "InstDrain"
            limit = 0 if is_drain and len(waits) > 1 else 1
            if len(waits) > limit:
                keep = waits[-limit:] if limit else []
                extra = waits[:-limit] if limit else waits
                for i0 in range(0, len(extra), nop_max):
                    n += 1
                    nop = mybir.InstNoOp(name=f"waitsplit_{n}", ins=[], outs=[])
                    nop.engine = ins.engine
                    nop.sync_info = mybir.SyncInfo(
                        on_wait=extra[i0 : i0 + nop_max], on_update=[]
                    )
                    out.append(nop)
                ins.sync_info = mybir.SyncInfo(
                    on_wait=keep, on_update=list(si.on_update or [])
                )
            out.append(ins)
        bb.instructions[:] = out
    return n


def build_nc(F=F_PTS, T=T_TILES, pool_split=True, split_waits=True):
    """Build the per-core Bass program. Same program on all 8 cores."""
    nc = bass.Bass()
    P = 128
    npts = P * F * T

    rot_d = nc.declare_dram_parameter("rotation", [npts, 4], F32, isOutput=False)
    scal_d = nc.declare_dram_parameter("scaling", [npts, 3], F32, isOutput=False)
    out_d = nc.declare_dram_parameter("symm", [npts, 6], F32, isOutput=True)

    with TileContext(nc) as tc:
        with (
            tc.tile_pool(name="io", bufs=2) as io,
            tc.tile_pool(name="sc", bufs=1) as sc,
            tc.tile_pool(name="mid", bufs=1) as mid,
        ):
            for t in range(T):
                rows = slice(t * P * F, (t + 1) * P * F)

                ROT = io.tile([P, 4 * F], F32, tag="rot")
                SCAL = io.tile([P, 3 * F], F32, tag="scal")
                OUT = io.tile([P, 6 * F], F32, tag="out")
                nc.sync.dma_start(
                    ROT[:], rot_d[rows, :].rearrange("(p f) c -> p (f c)", p=P)
                )
                nc.sync.dma_start(
                    SCAL[:], scal_d[rows, :].rearrange("(p f) c -> p (f c)", p=P)
                )

                # cross-engine tiles: double-buffered
                SIGI = sc.tile([P, 3 * F], BF16, tag="sigi")
                SIGP = sc.tile([P, 3 * F], BF16, tag="sigp")
                L = sc.tile([P, 9 * F], BF16, tag="l")
                LSQ = sc.tile([P, 9 * F], BF16, tag="lsq")
                PP = sc.tile([P, 9 * F], BF16, tag="pp")
                # mid tiles
                R2P = mid.tile([P, 4 * F], BF16, tag="r2p")
                SQP = mid.tile([P, 4 * F], BF16, tag="sqp")
                HADP = mid.tile([P, 4 * F], BF16, tag="hadp")
                N2 = mid.tile([P, F], F32, tag="n2")
                G = mid.tile([P, F], F32, tag="g")
                Y0 = mid.tile([P, F], F32, tag="y0")
                YM = mid.tile([P, F], BF16, tag="ym")
                GB = mid.tile([P, F], BF16, tag="gb")
                T3 = mid.tile([P, 3 * F], BF16, tag="t3")
                PRD = mid.tile([P, 6 * F], BF16, tag="prd")
                RU = mid.tile([P, 9 * F], BF16, tag="ru")
                VD = mid.tile([P, 3 * F], BF16, tag="vd")
                VO = mid.tile([P, 3 * F], BF16, tag="vo")

                ve = nc.vector
                se = nc.scalar
                pe = nc.gpsimd if pool_split else nc.vector

                # views: interleaved tiles walked (f, c); plane tiles as (c, f)
                def inter(tile, k):
                    return tile[:].rearrange("p (f c) -> p f c", c=k)

                def planes_fc(tile, k):
                    # plane tile walked in (f, c) order (matches interleaved)
                    return tile[:].rearrange("p (c f) -> p f c", f=F)

                def pl(tile, k):
                    # plane tile as (p, c, f)
                    return tile[:].rearrange("p (c f) -> p c f", f=F)

                # ---- ScalarE: sigmoid, dense in/out ----
                se.activation(SIGI[:], SCAL[:], ACTF.Sigmoid)

                # ---- DVE: AoS->plane conversions (single-src; dense writes,
                # strided reads -- scatter-write measured 2x slower) ----
                # R2P = rot planes [r x y z] in bf16
                ve.tensor_scalar(
                    pl(R2P, 4),
                    ROT[:].rearrange("p (f c) -> p c f", c=4),
                    1.0, None, ALU.mult,
                )
                ve.tensor_scalar(
                    pl(SIGP, 3),
                    SIGI[:].rearrange("p (f c) -> p c f", c=3),
                    1.0, None, ALU.mult,
                )

                rp = pl(R2P, 4)
                sq = pl(SQP, 4)   # planes: rr xx yy zz
                had = pl(HADP, 4)  # planes: pm p qm q
                ru = pl(RU, 9)
                prd = pl(PRD, 6)  # planes: pxy pyz pxz prz prx pry (doubled)

                # ---- squares on ScalarE (dense bf16) ----
                se.activation(SQP[:], R2P[:], ACTF.Square)

                # (p, q) = (rr+xx, yy+zz) -> HADP planes (1,3)
                ve.tensor_tensor(
                    had[:, 1:4:2, :], sq[:, 0:3:2, :], sq[:, 1:4:2, :], ALU.add
                )
                # (pm, qm)*2 -> HADP planes (0,2)
                ve.tensor_tensor(
                    had[:, 0:3:2, :], sq[:, 0:3:2, :], sq[:, 1:4:2, :], ALU.subtract
                )
                # n2 (fp32 for reciprocal)
                ve.tensor_tensor(
                    N2[:].unsqueeze(1), had[:, 1:2, :], had[:, 3:4, :], ALU.add
                )
                # D1 -> RU plane 1
                ve.tensor_tensor(
                    ru[:, 1:2, :], had[:, 0:1, :], had[:, 2:3, :], ALU.add
                )
                # (D2, D0) -> RU planes (5,6)
                ve.tensor_tensor(
                    ru[:, 5:7, :], had[:, 0:2, :], had[:, 2:4, :], ALU.subtract
                )
                # g ~= 1/n2 via bitwise-NOT seed + 1 Newton step (~2e-3 max
                # rel err): notx = bits(~n2); y0 = notx*c0;
                # ym = (n2*y0 - c1)*y0 = -y1;  GB = -A*ym = A/n2.
                ve.tensor_scalar(
                    G[:].bitcast(mybir.dt.uint32),
                    N2[:].bitcast(mybir.dt.uint32),
                    0xFFFFFFFF, None, ALU.bitwise_xor,
                )
                ve.tensor_scalar(Y0[:], G[:], -0.23549792, None, ALU.mult)
                ve.tensor_tensor(G[:], Y0[:], N2[:], ALU.mult)
                ve.scalar_tensor_tensor(
                    YM[:], G[:], 2.0017324, Y0[:], ALU.subtract, ALU.mult
                )
                ve.tensor_scalar(GB[:], YM[:], -A_SC, None, ALU.mult)
                # t_j = sigmoid_j * (A/(2*n2))  -> T3 planes (t0,t1,t2)
                t3 = pl(T3, 3)
                ve.tensor_tensor(
                    t3,
                    pl(SIGP, 3),
                    GB[:].unsqueeze(1).broadcast_to((P, 3, F)),
                    ALU.mult,
                )
                # doubled products: prd = 2*comp_a*comp_b (stt supplies the 2)
                ve.scalar_tensor_tensor(
                    prd[:, 0:2, :], rp[:, 1:3, :], 2.0, rp[:, 2:4, :],
                    ALU.mult, ALU.mult,
                )
                ve.scalar_tensor_tensor(
                    prd[:, 2:3, :], rp[:, 1:2, :], 2.0, rp[:, 3:4, :],
                    ALU.mult, ALU.mult,
                )
                ve.scalar_tensor_tensor(
                    prd[:, 4:6, :],
                    rp[:, 0:1, :].broadcast_to((P, 2, F)), 2.0,
                    rp[:, 1:3, :], ALU.mult, ALU.mult,
                )
                ve.scalar_tensor_tensor(
                    prd[:, 3:4, :], rp[:, 0:1, :], 2.0, rp[:, 3:4, :],
                    ALU.mult, ALU.mult,
                )
                # E sums: (E10,E21,E02) -> RU planes (0,4,8)
                ve.tensor_tensor(
                    ru[:, 0:9:4, :], prd[:, 0:3, :], prd[:, 3:6, :], ALU.add
                )
                # E diffs: (E12,E20) -> RU planes (2,3); E01 -> plane 7
                ve.tensor_tensor(
                    ru[:, 2:4, :], prd[:, 1:3, :], prd[:, 4:6, :], ALU.subtract
                )
                ve.tensor_tensor(
                    ru[:, 7:8, :], prd[:, 0:1, :], prd[:, 3:4, :], ALU.subtract
                )

                # ---- L = RU * t (t repeated per row-block) ----
                ru4 = RU[:].rearrange("p (i j f) -> p i j f", i=3, j=3)
                l4 = L[:].rearrange("p (i j f) -> p i j f", i=3, j=3)
                tpat = T3[:].rearrange("p (j f) -> p j f", j=3).unsqueeze(1)
                ve.tensor_tensor(
                    l4, ru4, tpat.broadcast_to((P, 3, 3, F)), ALU.mult
                )

                # ---- LSQ = L^2 on ScalarE (dense) ----
                se.activation(LSQ[:], L[:], ACTF.Square)

                # ---- PP: row-pair products (blocks: 0=row1, 1=row2, 2=row0)
                Lf = L[:]
                PPf = PP[:]
                pe.tensor_tensor(
                    PPf[:, 0 : 3 * F], Lf[:, 6 * F : 9 * F], Lf[:, 0 : 3 * F],
                    ALU.mult,
                )
                pe.tensor_tensor(
                    PPf[:, 3 * F : 6 * F], Lf[:, 6 * F : 9 * F],
                    Lf[:, 3 * F : 6 * F], ALU.mult,
                )
                pe.tensor_tensor(
                    PPf[:, 6 * F : 9 * F], Lf[:, 0 : 3 * F], Lf[:, 3 * F : 6 * F],
                    ALU.mult,
                )

                # ---- reductions over j (planes j0+j1, then +j2) ----
                lsq = pl(LSQ, 9)
                pp = pl(PP, 9)
                vd = pl(VD, 3)
                vo = pl(VO, 3)
                ve.tensor_tensor(vd, lsq[:, 0:9:3, :], lsq[:, 1:9:3, :], ALU.add)
                ve.tensor_tensor(vo, pp[:, 0:9:3, :], pp[:, 1:9:3, :], ALU.add)
                # final sums -> interleaved f32 OUT directly
                # diag: blocks (0,1,2) = (C11, C22, C00) -> OUT slots (3,5,0)
                outv = inter(OUT, 6)
                ve.tensor_tensor(
                    outv[:, :, 3:6:2],
                    planes_fc(VD, 3)[:, :, 0:2],
                    LSQ[:].rearrange("p (c f) -> p f c", f=F)[:, :, 2:6:3],
                    ALU.add,
                )
                ve.tensor_tensor(
                    outv[:, :, 0:1],
                    planes_fc(VD, 3)[:, :, 2:3],
                    LSQ[:].rearrange("p (c f) -> p f c", f=F)[:, :, 8:9],
                    ALU.add,
                )
                # off-diag: groups (C01, C02, C12) -> OUT slots (1,2,4)
                ve.tensor_tensor(
                    outv[:, :, 1:3],
                    planes_fc(VO, 3)[:, :, 0:2],
                    PP[:].rearrange("p (c f) -> p f c", f=F)[:, :, 2:6:3],
                    ALU.add,
                )
                ve.tensor_tensor(
                    outv[:, :, 4:5],
                    planes_fc(VO, 3)[:, :, 2:3],
                    PP[:].rearrange("p (c f) -> p f c", f=F)[:, :, 8:9],
                    ALU.add,
                )

                # ---- store ----
                nc.sync.dma_start(
                    out_d[rows, :].rearrange("(p f) c -> p (f c)", p=P), OUT[:]
                )
    if split_waits:
        _split_sync_waits(nc)
    return nc


_NC_CACHE = {}


def _get_nc(F, T, pool_split=True):
    key = (F, T, pool_split)
    if key not in _NC_CACHE:
        _NC_CACHE[key] = build_nc(F, T, pool_split)
    return _NC_CACHE[key]


def kernel(scaling: np.ndarray, rotation: np.ndarray) -> np.ndarray:
    from concourse.bass_utils import run_bass_kernel_spmd

    scaling = np.ascontiguousarray(np.asarray(scaling, dtype=np.float32))
    rotation = np.ascontiguousarray(np.asarray(rotation, dtype=np.float32))
    n = scaling.shape[0]

    ntot = N_CORES * P_CORE
    scal_p = np.zeros((ntot, 3), dtype=np.float32)
    rot_p = np.zeros((ntot, 4), dtype=np.float32)
    rot_p[:, 0] = 1.0  # benign quaternion for padding
    scal_p[:n] = scaling
    rot_p[:n] = rotation

    nc = _get_nc(F_PTS, T_TILES)
    in_maps = [
        {
            "scaling": scal_p[i * P_CORE : (i + 1) * P_CORE],
            "rotation": rot_p[i * P_CORE : (i + 1) * P_CORE],
        }
        for i in range(N_CORES)
    ]
    res = run_bass_kernel_spmd(nc, in_maps, list(range(N_CORES)))
    out = np.concatenate([res.results[i]["symm"] for i in range(N_CORES)], axis=0)
    return out[:n]


# revision 21
# speedup vs baseline: 1.5782x; 1.2035x over previous
"""Trainium2 Bass kernel: Gaussian-splat covariance from (scaling, rotation).

Math (per point n):
  s   = sigmoid(scaling)*(SMAX-SMIN) + SMIN   (SMIN dropped: <=1e-4 abs)
  q   = rotation / ||rotation||;  r,x,y,z = q
  R   = quaternion rotation matrix (3x3)
  L   = R @ diag(s);  C = L @ L^T;  out = upper-tri 6 of C

Implemented with unnormalized quaternions:  RU = n2*R,  G = 1/n2,
t = sigmoid*A*G,  L = RU*diag(t) = R*diag(s).

Layout strategy (the key to DVE throughput): all intermediates are bf16
component PLANES — tile [128, k*F] holds component c in a contiguous
F-long run per partition.  Every vector op streams long dense step-1 bf16
runs -> DVE 2x_1p packed mode; single-src ops get 2x_2p regardless of
stride, which makes the AoS<->plane conversions cheap tensor_scalar
copies.  ScalarE only runs dense-in/dense-out activations (its strided
writes measured ~4.5x slower).

Row/col permutation of R is chosen so that every E/D/partial-sum write is
a uniform-stride plane AP: slot(i,j) = 3*a_i + b_j, a=(2,0,1), b=id.

8-way data parallel over points; per core T tiles of 128*F points.
"""

import numpy as np

import concourse.bass as bass
import concourse.mybir as mybir
from concourse.tile import TileContext

F32 = mybir.dt.float32
BF16 = mybir.dt.bfloat16
ALU = mybir.AluOpType
ACTF = mybir.ActivationFunctionType

SCALE_MIN = 1e-4
SCALE_MAX = 10.0
A_SC = SCALE_MAX - SCALE_MIN

N_CORES = 8
N_TOTAL = 4_000_000

# Per-core tiling: P_CORE = 128 * F * T points.
F_PTS = 560
T_TILES = 7
P_CORE = 128 * F_PTS * T_TILES  # 501760; 8 cores cover 4,014,080 >= 4e6


def _split_sync_waits(nc, nop_max=1):
    """This container's walrus encodes at most 2 sync waits per instruction
    (and none on Drain). Move excess waits onto dedicated NoOps upstream."""
    n = 0
    for bb in nc.main_func.blocks:
        out = []
        for ins in bb.instructions:
            si = ins.sync_info
            waits = list(si.on_wait) if (si is not None and si.on_wait) else []
            is_drain = type(ins).__name__ == "InstDrain"
            limit = 0 if is_drain and len(waits) > 1 else 1
            if len(waits) > limit:
                keep = waits[-limit:] if limit else []
                extra = waits[:-limit] if limit else waits
                for i0 in range(0, len(extra), nop_max):
                    n += 1
                    nop = mybir.InstNoOp(name=f"waitsplit_{n}", ins=[], outs=[])
                    nop.engine = ins.engine
                    nop.sync_info = mybir.SyncInfo(
                        on_wait=extra[i0 : i0 + nop_max], on_update=[]
                    )
                    out.append(nop)
                ins.sync_info = mybir.SyncInfo(
                    on_wait=keep, on_update=list(si.on_update or [])
                )
            out.append(ins)
        bb.instructions[:] = out
    return n


def build_nc(F=F_PTS, T=T_TILES, pool_split=True, split_waits=True):
    """Build the per-core Bass program. Same program on all 8 cores."""
    nc = bass.Bass()
    P = 128
    npts = P * F * T

    rot_d = nc.declare_dram_parameter("rotation", [npts, 4], F32, isOutput=False)
    scal_d = nc.declare_dram_parameter("scaling", [npts, 3], F32, isOutput=False)
    out_d = nc.declare_dram_parameter("symm", [npts, 6], F32, isOutput=True)

    with TileContext(nc) as tc:
        with (
            tc.tile_pool(name="io", bufs=2) as io,
            tc.tile_pool(name="sc", bufs=2) as sc,
            tc.tile_pool(name="mid", bufs=1) as mid,
        ):
            for t in range(T):
                rows = slice(t * P * F, (t + 1) * P * F)

                ROT = io.tile([P, 4 * F], F32, tag="rot")
                SCAL = io.tile([P, 3 * F], F32, tag="scal")
                OUT = io.tile([P, 6 * F], F32, tag="out")
                nc.sync.dma_start(
                    ROT[:], rot_d[rows, :].rearrange("(p f) c -> p (f c)", p=P)
                )
                nc.sync.dma_start(
                    SCAL[:], scal_d[rows, :].rearrange("(p f) c -> p (f c)", p=P)
                )

                # cross-engine tiles: double-buffered
                SIGI = sc.tile([P, 3 * F], BF16, tag="sigi")
                SIGP = sc.tile([P, 3 * F], BF16, tag="sigp")
                L = sc.tile([P, 9 * F], BF16, tag="l")
                LSQ = sc.tile([P, 9 * F], BF16, tag="lsq")
                PP = sc.tile([P, 9 * F], BF16, tag="pp")
                # mid tiles
                P2 = mid.tile([P, 4 * F], BF16, tag="p2")
                SQP = mid.tile([P, 4 * F], BF16, tag="sqp")
                HADP = mid.tile([P, 4 * F], BF16, tag="hadp")
                N2 = mid.tile([P, F], F32, tag="n2")
                G = mid.tile([P, F], F32, tag="g")
                Y0 = mid.tile([P, F], F32, tag="y0")
                YM = mid.tile([P, F], BF16, tag="ym")
                GB = mid.tile([P, F], BF16, tag="gb")
                T3 = mid.tile([P, 3 * F], BF16, tag="t3")
                PRD = mid.tile([P, 6 * F], BF16, tag="prd")
                RU = mid.tile([P, 9 * F], BF16, tag="ru")
                VD = mid.tile([P, 3 * F], BF16, tag="vd")
                VO = mid.tile([P, 3 * F], BF16, tag="vo")
                RES = mid.tile([P, 6 * F], BF16, tag="res")

                ve = nc.vector
                se = nc.scalar
                pe = nc.gpsimd if pool_split else nc.vector

                # views: interleaved tiles walked (f, c); plane tiles as (c, f)
                def inter(tile, k):
                    return tile[:].rearrange("p (f c) -> p f c", c=k)

                def planes_fc(tile, k):
                    # plane tile walked in (f, c) order (matches interleaved)
                    return tile[:].rearrange("p (c f) -> p f c", f=F)

                def pl(tile, k):
                    # plane tile as (p, c, f)
                    return tile[:].rearrange("p (c f) -> p c f", f=F)

                # ---- front-end: ScalarE dense sigmoid; DVE scatter-read
                # copies deposit bf16 planes ----
                se.activation(SIGI[:], SCAL[:], ACTF.Sigmoid)
                # P2 = 2*rot planes [2r 2x 2y 2z]
                ve.tensor_scalar(
                    pl(P2, 4),
                    ROT[:].rearrange("p (f c) -> p c f", c=4),
                    2.0, None, ALU.mult,
                )
                ve.tensor_scalar(
                    pl(SIGP, 3),
                    SIGI[:].rearrange("p (f c) -> p c f", c=3),
                    1.0, None, ALU.mult,
                )
                # SQP = ((2a)/sqrt2)^2 = 2a^2: planes 2rr 2xx 2yy 2zz
                se.activation(SQP[:], P2[:], ACTF.Square, scale=0.7071067811865476)

                q2 = pl(P2, 4)
                sq = pl(SQP, 4)   # planes: 2rr 2xx 2yy 2zz
                had = pl(HADP, 4)  # planes: pm p qm q      (2x scale)
                ru = pl(RU, 9)     # = 2*Ru
                prd = pl(PRD, 6)  # planes: pxy pyz pxz prz prx pry (2x scale)

                # (p, q) = (rr+xx, yy+zz) -> HADP planes (1,3)
                ve.tensor_tensor(
                    had[:, 1:4:2, :], sq[:, 0:3:2, :], sq[:, 1:4:2, :], ALU.add
                )
                # (pm, qm)*2 -> HADP planes (0,2)
                ve.tensor_tensor(
                    had[:, 0:3:2, :], sq[:, 0:3:2, :], sq[:, 1:4:2, :], ALU.subtract
                )
                # n2 (fp32 for reciprocal)
                ve.tensor_tensor(
                    N2[:].unsqueeze(1), had[:, 1:2, :], had[:, 3:4, :], ALU.add
                )
                # D1 -> RU plane 1
                ve.tensor_tensor(
                    ru[:, 1:2, :], had[:, 0:1, :], had[:, 2:3, :], ALU.add
                )
                # (D2, D0) -> RU planes (5,6)
                ve.tensor_tensor(
                    ru[:, 5:7, :], had[:, 0:2, :], had[:, 2:4, :], ALU.subtract
                )
                # g ~= 1/n2 via bitwise-NOT seed + 1 Newton step (~2e-3 max
                # rel err): notx = bits(~n2); y0 = notx*c0;
                # ym = (n2*y0 - c1)*y0 = -y1;  GB = -A*ym = A/n2.
                ve.tensor_scalar(
                    G[:].bitcast(mybir.dt.uint32),
                    N2[:].bitcast(mybir.dt.uint32),
                    0xFFFFFFFF, None, ALU.bitwise_xor,
                )
                ve.tensor_scalar(Y0[:], G[:], -0.23549792, None, ALU.mult)
                ve.tensor_tensor(G[:], Y0[:], N2[:], ALU.mult)
                ve.scalar_tensor_tensor(
                    YM[:], G[:], 2.0017324, Y0[:], ALU.subtract, ALU.mult
                )
                ve.tensor_scalar(GB[:], YM[:], -A_SC, None, ALU.mult)
                # t_j = sigmoid_j * (A/(2*n2))  -> T3 planes (t0,t1,t2)
                t3 = pl(T3, 3)
                ve.tensor_tensor(
                    t3,
                    pl(SIGP, 3),
                    GB[:].unsqueeze(1).broadcast_to((P, 3, F)),
                    ALU.mult,
                )
                # doubled products: (2a)(2b) = 2*(2ab), matching RU scale
                ve.tensor_tensor(prd[:, 0:2, :], q2[:, 1:3, :], q2[:, 2:4, :], ALU.mult)
                ve.tensor_tensor(prd[:, 2:3, :], q2[:, 1:2, :], q2[:, 3:4, :], ALU.mult)
                ve.tensor_tensor(
                    prd[:, 4:6, :],
                    q2[:, 0:1, :].broadcast_to((P, 2, F)),
                    q2[:, 1:3, :], ALU.mult,
                )
                ve.tensor_tensor(prd[:, 3:4, :], q2[:, 0:1, :], q2[:, 3:4, :], ALU.mult)
                # E sums: (E10,E21,E02) -> RU planes (0,4,8)
                ve.tensor_tensor(
                    ru[:, 0:9:4, :], prd[:, 0:3, :], prd[:, 3:6, :], ALU.add
                )
                # E diffs: (E12,E20) -> RU planes (2,3); E01 -> plane 7
                ve.tensor_tensor(
                    ru[:, 2:4, :], prd[:, 1:3, :], prd[:, 4:6, :], ALU.subtract
                )
                ve.tensor_tensor(
                    ru[:, 7:8, :], prd[:, 0:1, :], prd[:, 3:4, :], ALU.subtract
                )

                # ---- L = RU * t (t repeated per row-block) ----
                ru4 = RU[:].rearrange("p (i j f) -> p i j f", i=3, j=3)
                l4 = L[:].rearrange("p (i j f) -> p i j f", i=3, j=3)
                tpat = T3[:].rearrange("p (j f) -> p j f", j=3).unsqueeze(1)
                ve.tensor_tensor(
                    l4, ru4, tpat.broadcast_to((P, 3, 3, F)), ALU.mult
                )

                # ---- LSQ = L^2 on ScalarE (dense) ----
                se.activation(LSQ[:], L[:], ACTF.Square)

                # ---- PP: row-pair products (blocks: 0=row1, 1=row2, 2=row0)
                Lf = L[:]
                PPf = PP[:]
                pe.tensor_tensor(
                    PPf[:, 0 : 3 * F], Lf[:, 6 * F : 9 * F], Lf[:, 0 : 3 * F],
                    ALU.mult,
                )
                pe.tensor_tensor(
                    PPf[:, 3 * F : 6 * F], Lf[:, 6 * F : 9 * F],
                    Lf[:, 3 * F : 6 * F], ALU.mult,
                )
                pe.tensor_tensor(
                    PPf[:, 6 * F : 9 * F], Lf[:, 0 : 3 * F], Lf[:, 3 * F : 6 * F],
                    ALU.mult,
                )

                # ---- reductions over j (planes j0+j1, then +j2) ----
                lsq = pl(LSQ, 9)
                pp = pl(PP, 9)
                vd = pl(VD, 3)
                vo = pl(VO, 3)
                ve.tensor_tensor(vd, lsq[:, 0:9:3, :], lsq[:, 1:9:3, :], ALU.add)
                ve.tensor_tensor(vo, pp[:, 0:9:3, :], pp[:, 1:9:3, :], ALU.add)
                # final sums -> RES planes (C00 C01 C02 C11 C12 C22), dense
                res = pl(RES, 6)
                # diag: blocks (0,1,2) = (C11, C22, C00) -> RES planes (3,5,0)
                ve.tensor_tensor(
                    res[:, 3:6:2, :], vd[:, 0:2, :], lsq[:, 2:6:3, :], ALU.add
                )
                ve.tensor_tensor(
                    res[:, 0:1, :], vd[:, 2:3, :], lsq[:, 8:9, :], ALU.add
                )
                # off-diag: groups (C01, C02, C12) -> RES planes (1,2,4)
                ve.tensor_tensor(
                    res[:, 1:3, :], vo[:, 0:2, :], pp[:, 2:6:3, :], ALU.add
                )
                ve.tensor_tensor(
                    res[:, 4:5, :], vo[:, 2:3, :], pp[:, 8:9, :], ALU.add
                )
                # plane -> interleaved f32 (strided read, dense write)
                ve.tensor_scalar(
                    inter(OUT, 6), planes_fc(RES, 6), 1.0, None, ALU.mult
                )

                # ---- store ----
                nc.sync.dma_start(
                    out_d[rows, :].rearrange("(p f) c -> p (f c)", p=P), OUT[:]
                )
    if split_waits:
        _split_sync_waits(nc)
    return nc


_NC_CACHE = {}


def _get_nc(F, T, pool_split=True):
    key = (F, T, pool_split)
    if key not in _NC_CACHE:
        _NC_CACHE[key] = build_nc(F, T, pool_split)
    return _NC_CACHE[key]


def kernel(scaling: np.ndarray, rotation: np.ndarray) -> np.ndarray:
    from concourse.bass_utils import run_bass_kernel_spmd

    scaling = np.ascontiguousarray(np.asarray(scaling, dtype=np.float32))
    rotation = np.ascontiguousarray(np.asarray(rotation, dtype=np.float32))
    n = scaling.shape[0]

    ntot = N_CORES * P_CORE
    scal_p = np.zeros((ntot, 3), dtype=np.float32)
    rot_p = np.zeros((ntot, 4), dtype=np.float32)
    rot_p[:, 0] = 1.0  # benign quaternion for padding
    scal_p[:n] = scaling
    rot_p[:n] = rotation

    nc = _get_nc(F_PTS, T_TILES)
    in_maps = [
        {
            "scaling": scal_p[i * P_CORE : (i + 1) * P_CORE],
            "rotation": rot_p[i * P_CORE : (i + 1) * P_CORE],
        }
        for i in range(N_CORES)
    ]
    res = run_bass_kernel_spmd(nc, in_maps, list(range(N_CORES)))
    out = np.concatenate([res.results[i]["symm"] for i in range(N_CORES)], axis=0)
    return out[:n]
